# revision 22
# baseline (speedup 1.0000x reference)
"""Trainium2 Bass kernel for a causal attention block (RMSNorm + fused QKV +
RoPE + causal attention + output projection).

Full (unsharded) inputs in, full output out.  Internally shards across 8
NeuronCores: data-parallel over batch (2) x tensor-parallel over heads
(4 groups of 4 heads).  Each core computes a partial output-projection
(contraction over its 512 head-dims); the host sums the 4 partials per batch
and adds o_b.

v2: all matmul inputs in bf16 (full-rate PE streaming, no fp32r small-tile
penalty), XBAR DMA transposes instead of PE transposes, q/k/v kept SBUF
resident (no DRAM round trip), and a 512-token-chunk software pipeline:
RMSNorm(g) -> QKV+RoPE(g) -> attention(g) -> o_proj(g) with RMSNorm(g+1)
interleaved into attention(g).  Partial outputs are written bf16; the host
accumulates in fp32.
"""

import os
import sys

import numpy as np

for _p in ("/opt/trn_rl_repo", "/opt/pypackages"):
    if _p not in sys.path and os.path.isdir(_p):
        sys.path.append(_p)

import ml_dtypes
import concourse.mybir as mybir
import concourse.tile as tile
from concourse import bacc
from concourse.bass_utils import run_bass_kernel_spmd

F32 = mybir.dt.float32
BF16 = mybir.dt.bfloat16
AF = mybir.ActivationFunctionType

B, T, C = 2, 2048, 2048
H, D = 16, 128
EPS = 1e-5
ROPE_BASE = 10000.0
NCORES = 8
HPC = 4          # heads per core
MB = 3 * HPC     # qkv m-blocks per core (12)
KC = C // 128    # 16 contraction blocks
NT = T // 512    # 4 t-chunks
TBL = T // 128   # 16 t-blocks
SCALE = 1.0 / float(np.sqrt(D))

last_exec_time_ns = None
last_result = None
_cache = {}


def _build_nc(debug_dumps=False):
    nc = bacc.Bacc("TRN2", target_bir_lowering=False, debug=False,
                   num_devices=NCORES)
    x_d = nc.declare_dram_parameter("x", [T, C], BF16, isOutput=False)
    w_d = nc.declare_dram_parameter("wqkv", [128, MB, KC, 128], BF16,
                                    isOutput=False)
    b_d = nc.declare_dram_parameter("bqkv", [128, MB], F32, isOutput=False)
    ow_d = nc.declare_dram_parameter("owT", [128, HPC, C], BF16,
                                     isOutput=False)
    cos_d = nc.declare_dram_parameter("cosT", [128, T], BF16, isOutput=False)
    sin_d = nc.declare_dram_parameter("sinmT", [128, T], BF16, isOutput=False)
    tri_d = nc.declare_dram_parameter("trimask", [128, 128], F32,
                                      isOutput=False)
    on_d = nc.declare_dram_parameter("ones_w", [128, 1], BF16, isOutput=False)
    out_d = nc.declare_dram_parameter("out", [T, C], BF16, isOutput=True)

    xn_d = nc.dram_tensor("xn_scratch", [T, C], BF16)
    v_d = nc.dram_tensor("v_scratch", [128, HPC, T], BF16)
    if debug_dumps:
        dbg_xnT = nc.declare_dram_parameter("dbg_xnT", [NT, 128, KC, 512],
                                            BF16, isOutput=True)
        dbg_k = nc.declare_dram_parameter("dbg_k", [128, T], BF16,
                                          isOutput=True)
        dbg_v = nc.declare_dram_parameter("dbg_v", [128, TBL, 128], BF16,
                                          isOutput=True)
        dbg_q = nc.declare_dram_parameter("dbg_q", [128, 512], BF16,
                                          isOutput=True)
        dbg_aT = nc.declare_dram_parameter("dbg_aT", [128, HPC, 512], BF16,
                                           isOutput=True)
        dbg_xt = nc.declare_dram_parameter("dbg_xt", [128, C], BF16,
                                           isOutput=True)
        dbg_xs = nc.declare_dram_parameter("dbg_xs", [128, C], BF16,
                                           isOutput=True)
        dbg_wq = nc.declare_dram_parameter("dbg_wq", [128, KC, 128], BF16,
                                           isOutput=True)
        dbg_xnd = nc.declare_dram_parameter("dbg_xnd", [512, C], BF16,
                                            isOutput=True)

    with tile.TileContext(nc) as tc:
        with tc.tile_pool(name="const", bufs=1) as constp:
            # ---- resident constants / weights --------------------------
            cosT = constp.tile([128, T], BF16, tag="cosT")
            nc.sync.dma_start(out=cosT, in_=cos_d[:, :])
            sinmT = constp.tile([128, T], BF16, tag="sinmT")
            nc.sync.dma_start(out=sinmT, in_=sin_d[:, :])
            trimask = constp.tile([128, 128], F32, tag="trimask")
            nc.sync.dma_start(out=trimask, in_=tri_d[:, :])
            bqkv = constp.tile([128, MB], F32, tag="bqkv")
            nc.sync.dma_start(out=bqkv, in_=b_d[:, :])
            ones_w = constp.tile([128, 1], BF16, tag="ones_w")
            nc.sync.dma_start(out=ones_w, in_=on_d[:, :])
            epst = constp.tile([128, 1], F32, tag="epst")
            nc.vector.memset(epst, EPS)
            wq = constp.tile([128, MB, KC, 128], BF16, tag="wq")
            owT = constp.tile([128, HPC, C], BF16, tag="owT")

            # ---- resident k/v (filled by stage B, read by stage C) -----
            # q of chunk g is only read by attention of chunk g, so q lives
            # in a small per-chunk ring instead of full-T residency.
            kTs, vtrs = [], []
            for h in range(HPC):
                kT = constp.tile([128, T], BF16, tag=f"kT{h}", name=f"kT{h}")
                vtr = constp.tile([128, TBL, 128], BF16, tag=f"vtr{h}",
                                  name=f"vtr{h}")
                kTs.append(kT)
                vtrs.append(vtr)

            with (tc.tile_pool(name="xtp", bufs=2) as xtp,
                  tc.tile_pool(name="xsp", bufs=2) as xsp,
                  tc.tile_pool(name="stat", bufs=4) as stat,
                  tc.tile_pool(name="xnTp", bufs=2) as xnTp,
                  tc.tile_pool(name="qsp", bufs=3) as qsp,
                  tc.tile_pool(name="qswp", bufs=2) as qswp,
                  tc.tile_pool(name="qTp", bufs=2) as qTp,
                  tc.tile_pool(name="ptp", bufs=3) as ptp,
                  tc.tile_pool(name="rcp", bufs=2) as rcp,
                  tc.tile_pool(name="aTp", bufs=2) as aTp,
                  tc.tile_pool(name="ostp", bufs=2) as ostp,
                  tc.tile_pool(name="mm512", bufs=2, space="PSUM") as mmp,
                  tc.tile_pool(name="scpp", bufs=2, space="PSUM") as scpp,
                  tc.tile_pool(name="uop", bufs=2, space="PSUM") as uop,
                  tc.tile_pool(name="rsp", bufs=2, space="PSUM") as rsp):

                # PE warm-up: keep the array busy during stage A(0) so the
                # p-state is fully ramped when QKV starts.
                wup = rsp.tile([1, 512], F32, tag="rs_ps")
                for _ in range(10):
                    nc.tensor.matmul(wup, ones_w, cosT[:, 0:512],
                                     start=True, stop=True)

                def stage_a_block(tb):
                    """RMSNorm + bf16 cast + XBAR transpose of t-block tb."""
                    g, i = tb // 4, tb % 4
                    xt = xtp.tile([128, C], BF16, tag="xt")
                    nc.sync.dma_start(out=xt,
                                      in_=x_d[tb * 128:(tb + 1) * 128, :])
                    xsq = xsp.tile([128, C], BF16, tag="xsq")
                    ms = stat.tile([128, 1], F32, tag="ms")
                    # ms = sum((x/sqrt(C))^2) = mean(x^2)
                    nc.scalar.activation(out=xsq, in_=xt, func=AF.Square,
                                         scale=float(1.0 / np.sqrt(C)),
                                         accum_out=ms)
                    sd = stat.tile([128, 1], F32, tag="sd")
                    nc.scalar.activation(out=sd, in_=ms, func=AF.Sqrt,
                                         bias=epst, scale=1.0)
                    rs = stat.tile([128, 1], F32, tag="rs")
                    nc.vector.reciprocal(out=rs, in_=sd)
                    xs = xsp.tile([128, C], BF16, tag="xs")
                    nc.vector.tensor_scalar_mul(xs, xt, rs)
                    if debug_dumps and tb == 0:
                        nc.sync.dma_start(out=dbg_xt[:, :], in_=xt)
                        nc.sync.dma_start(out=dbg_xs[:, :], in_=xs)
                        nc.sync.dma_start(out=dbg_wq[:, :, :], in_=wq[:, 0, :, :])
                    nc.gpsimd.dma_start(out=xn_d[tb * 128:(tb + 1) * 128, :],
                                        in_=xs)
                    if i == 3:
                        nc.sync.dma_start_transpose(
                            out=xnT_g[g % 2],
                            in_=xn_d[g * 512:(g + 1) * 512, :])

                # double-buffered per-chunk xnT
                xnT_g = [xnTp.tile([128, KC, 512], BF16, tag="xnT",
                                   name=f"xnT{j}") for j in range(2)]

                for tb in range(4):          # stage A for chunk 0
                    stage_a_block(tb)
                # weight loads AFTER the chunk-0 x loads so stage A is not
                # queued behind 8.4MB of weight DMA; per-m so QKV m=0 can
                # start as soon as its slice lands.
                for m_ in [sec * HPC + h for h in range(HPC)
                           for sec in range(3)]:
                    nc.sync.dma_start(out=wq[:, m_, :, :],
                                      in_=w_d[:, m_, :, :])
                nc.sync.dma_start(out=owT, in_=ow_d[:, :, :])

                for g in range(NT):
                    nsl = slice(g * 512, (g + 1) * 512)
                    xn = xnT_g[g % 2]
                    if debug_dumps:
                        nc.sync.dma_start(out=dbg_xnT[g, :, :, :], in_=xn)
                        if g == 0:
                            nc.sync.dma_start(out=dbg_xnd[:, :],
                                              in_=xn_d[0:512, :])
                    # -------- stage B: QKV + bias + RoPE / v-transpose ----
                    qTc = []
                    for h in range(HPC):
                        for sec in range(3):      # q, k, v for head h
                            m = sec * HPC + h
                            ps = mmp.tile([128, 512], F32, tag="mm512")
                            for kk in range(KC):
                                nc.tensor.matmul(ps, wq[:, m, kk, :],
                                                 xn[:, kk, :],
                                                 start=(kk == 0),
                                                 stop=(kk == KC - 1))
                            qs = qsp.tile([128, 512], BF16, tag="qs")
                            nc.scalar.activation(out=qs, in_=ps,
                                                 func=AF.Identity,
                                                 bias=bqkv[:, m:m + 1],
                                                 scale=1.0)
                            if sec < 2:
                                if sec == 0:
                                    dst = qTp.tile([128, 512], BF16,
                                                   tag=f"qT{h}",
                                                   name=f"qTc{h}")
                                    dsl = slice(0, 512)
                                    qTc.append(dst)
                                else:
                                    dst = kTs[h]
                                    dsl = nsl
                                qsw = qswp.tile([128, 512], BF16, tag="qsw")
                                nc.scalar.dma_start(out=qsw[0:64, :],
                                                    in_=qs[64:128, :])
                                nc.scalar.dma_start(out=qsw[64:128, :],
                                                    in_=qs[0:64, :])
                                nc.vector.tensor_mul(qsw, qsw, sinmT[:, nsl])
                                nc.vector.tensor_mul(dst[:, dsl], qs,
                                                     cosT[:, nsl])
                                nc.vector.tensor_add(dst[:, dsl],
                                                     dst[:, dsl], qsw)
                            else:
                                nc.scalar.dma_start(out=v_d[:, h, nsl],
                                                    in_=qs)
                                nc.sync.dma_start_transpose(
                                    out=vtrs[h][:, g * 4:(g + 1) * 4, :],
                                    in_=v_d[:, h, nsl])
                        if sec == 2 and g < NT - 1:
                            stage_a_block((g + 1) * 4 + h)

                    if debug_dumps and g == 0:
                        nc.sync.dma_start(out=dbg_q[:, :], in_=qTc[0])

                    # -------- stage C: attention rows g*512..+512 ---------
                    aT = aTp.tile([128, HPC, 512], BF16, tag="aT")
                    for h in range(HPC):
                        qT, kT, vtr = qTc[h], kTs[h], vtrs[h]
                        uo = uop.tile([128, 512], F32, tag="uo")
                        rs_ps = rsp.tile([1, 512], F32, tag="rs_ps")
                        nj = 4 * g + 4
                        for jb in range(nj):
                            r = jb - 4 * g
                            u0 = 128 * r if r >= 0 else 0
                            usl = slice(u0, 512)
                            scp = scpp.tile([128, 512], F32, tag="scp")
                            nc.tensor.matmul(
                                scp[:, usl],
                                kT[:, jb * 128:(jb + 1) * 128],
                                qT[:, usl],
                                start=True, stop=True)
                            if r >= 0:
                                nc.vector.tensor_add(
                                    scp[:, u0:u0 + 128],
                                    scp[:, u0:u0 + 128], trimask)
                            pt = ptp.tile([128, 512], BF16, tag="pt")
                            nc.scalar.activation(out=pt[:, usl],
                                                 in_=scp[:, usl],
                                                 func=AF.Exp, scale=SCALE)
                            nc.tensor.matmul(uo[:, usl], vtr[:, jb, :],
                                             pt[:, usl],
                                             start=(jb == 0),
                                             stop=(jb == nj - 1))
                            nc.tensor.matmul(rs_ps[:, usl], ones_w,
                                             pt[:, usl],
                                             start=(jb == 0),
                                             stop=(jb == nj - 1))
                        rcs = rcp.tile([1, 512], F32, tag="rcs")
                        nc.vector.reciprocal_approx_fast(out=rcs, in_=rs_ps)
                        rb = rcp.tile([128, 512], F32, tag="rb")
                        nc.gpsimd.partition_broadcast(rb, rcs)
                        nc.vector.tensor_mul(aT[:, h, :], uo, rb)

                    if debug_dumps and g == 0:
                        nc.sync.dma_start(out=dbg_aT[:, :, :], in_=aT)
                    if debug_dumps and g == NT - 1:
                        nc.sync.dma_start(out=dbg_k[:, :], in_=kTs[0])
                        nc.sync.dma_start(out=dbg_v[:, :, :], in_=vtrs[0])

                    # -------- stage D: o_proj rows of this chunk ----------
                    for i in range(4):
                        tb = g * 4 + i
                        lsl = slice(i * 128, (i + 1) * 128)
                        ost = ostp.tile([128, C], BF16, tag="ost")
                        for n in range(NT):
                            pso = mmp.tile([128, 512], F32, tag="mm512")
                            for cb in range(HPC):
                                nc.tensor.matmul(
                                    pso, aT[:, cb, lsl],
                                    owT[:, cb, n * 512:(n + 1) * 512],
                                    start=(cb == 0), stop=(cb == HPC - 1))
                            osl = ost[:, n * 512:(n + 1) * 512]
                            if n % 2 == 0:
                                nc.vector.tensor_copy(osl, pso)
                            else:
                                nc.scalar.copy(osl, pso)
                        nc.gpsimd.dma_start(
                            out=out_d[tb * 128:(tb + 1) * 128, :], in_=ost)

    nc.compile()
    return nc


def _get_nc():
    if "nc" not in _cache:
        _cache["nc"] = _build_nc()
    return _cache["nc"]


def _host_prep(x, rms_weight, qkv_w, qkv_b, o_w):
    """Build the per-core input maps."""
    x = np.asarray(x, dtype=np.float32)
    rms_weight = np.asarray(rms_weight, dtype=np.float32)
    qkv_w = np.asarray(qkv_w, dtype=np.float32)
    qkv_b = np.asarray(qkv_b, dtype=np.float32)
    o_w = np.asarray(o_w, dtype=np.float32)

    w_eff = qkv_w * rms_weight[None, :]

    pos = np.arange(T, dtype=np.float32)
    inv_freq = (1.0 / (ROPE_BASE ** (np.arange(0, D, 2, dtype=np.float32)
                                     / D))).astype(np.float32)
    F = pos[:, None] * inv_freq[None, :]          # [T, 64]
    cos_td = np.cos(F).astype(np.float32)
    sin_td = np.sin(F).astype(np.float32)
    cosT = np.ascontiguousarray(
        np.concatenate([cos_td.T, cos_td.T], axis=0)).astype(
        ml_dtypes.bfloat16)                        # [128, T]
    sinmT = np.ascontiguousarray(
        np.concatenate([-sin_td.T, sin_td.T], axis=0)).astype(
        ml_dtypes.bfloat16)                        # [128, T]

    iu = np.arange(128)
    trimask = np.where(iu[None, :] >= iu[:, None], 0.0,
                       -1e30).astype(np.float32)

    in_maps = []
    for core in range(NCORES):
        b = core // 4
        g = core % 4
        rows = np.concatenate([
            np.arange(sec * C + g * 512, sec * C + (g + 1) * 512)
            for sec in range(3)])
        shard = w_eff[rows]                       # [1536, C]
        wqkv = np.ascontiguousarray(
            shard.reshape(MB, 128, KC, 128).transpose(3, 0, 2, 1)).astype(
            ml_dtypes.bfloat16)
        bqkv = np.ascontiguousarray(
            qkv_b[rows].reshape(MB, 128).T)       # [128, MB]
        owT = np.ascontiguousarray(
            o_w[:, g * 512:(g + 1) * 512].reshape(C, HPC, 128)
            .transpose(2, 1, 0)).astype(ml_dtypes.bfloat16)  # [128, HPC, C]
        in_maps.append({
            "x": np.ascontiguousarray(x[b]).astype(ml_dtypes.bfloat16),
            "wqkv": wqkv,
            "bqkv": bqkv,
            "owT": owT,
            "cosT": cosT,
            "sinmT": sinmT,
            "trimask": trimask,
            "ones_w": np.ones((128, 1), dtype=ml_dtypes.bfloat16),
        })
    return in_maps


def kernel(x, rms_weight, qkv_w, qkv_b, o_w, o_b):
    global last_exec_time_ns
    o_b = np.asarray(o_b, dtype=np.float32)
    in_maps = _host_prep(x, rms_weight, qkv_w, qkv_b, o_w)
    nc = _get_nc()

    trace = bool(int(os.environ.get("BASSK_TRACE", "0")))
    if trace:
        try:
            import ntff_shim
            ntff_shim.install()
        except Exception:
            pass
    res = None
    for attempt in range(3):
        try:
            res = run_bass_kernel_spmd(nc, in_maps, list(range(NCORES)),
                                       trace=trace)
            break
        except Exception:
            if attempt == 2:
                raise
            import time
            time.sleep(5)
    last_exec_time_ns = res.exec_time_ns
    globals()["last_result"] = res

    out = np.empty((B, T, C), dtype=np.float32)
    for b in range(B):
        acc = res.results[4 * b]["out"].astype(np.float32)
        for g in range(1, 4):
            acc += res.results[4 * b + g]["out"].astype(np.float32)
        out[b] = acc + o_b[None, :]
    return out
